# revision 25
# baseline (speedup 1.0000x reference)
"""Trainium2 Bass kernel for nn_MemoryEngineLayer (scatter_memory).

Contract: kernel(**inputs) takes FULL unsharded inputs (as produced by
setup_inputs()) and returns the FULL [B, T, H] output.

Sharding: sequence-parallel with warmup. Core c handles the T-window
[c*512, (c+1)*512) for ALL B=8 batch sequences. Each window is preceded by
W=128 warmup steps (real x for c>0; zeros for c=0 — with zero input the
recurrence state (v0, v0) is an exact fixed point: renorm((g+gp)*v0) = v0,
so core 0's math is exact). The top-k injection is state-independent, and
the gamma=0.92 contraction + renorm forgets the initial state: warmup of
128 steps leaves ~5e-3 relative tape error at the window start, decaying to
~1e-5 within 256 steps; the y-output is dominated by the x passthrough
(tape term is ~0.4% of ||y||), so the global rel err stays ~1e-5.

Why: the scan's serial chain (DVE stt -> PE partition-reduce -> ACT rsqrt ->
next step's DVE) is latency-bound at ~2.2us/step when one chain runs alone.
With all 8 batch chains interleaved per core, each engine processes 8
independent steps' work back-to-back and the cross-engine latency is hidden;
throughput becomes issue-bound (~0.3us/step).

Math per chain (normalized-tape form; equivalent to the reference):
  m'_t   = x_t @ (beta/gamma * basis[:, :256])           # [256], top-8 inject
  inj'_t = where(|m'_t| >= kth8(|m'_t|), m'_t, 0)
  u_t    = tape_{t-1} + (g/gamma) tape_{t-2} + inj'_t     (complex 256)
  r_t    = rsqrt(||u_t||^2)
  tape_t = r_t u_t
  y_t    = x_t + Re(tape_t) @ (alpha * bg * basis[:, :256]).T
Carrying the normalized tape as state removes the r_{t-1}/gr scalar chains
from the vector recurrence entirely: only tape values cross steps.

On-chip layout: complex state as [128 partitions, 4 cols] = (re_lo, re_hi,
im_lo, im_hi) with slot s = q*128 + p; the per-step state lives at cols
4t..4t+4 of the per-chunk Z buffer (contiguous, also the y-matmul source).
Per macro-step, all 8 chains share ONE PE ones-matmul partition-reduce
([128,8]) and ONE batched ACT rsqrt producing r for all chains at once.
Engine budget per chain-step: DVE stt tmp (~123ns) + DVE accum d (~111ns)
+ Pool tensor_tensor u (~138ns) + tape write (ACT ~281ns / DVE ~144ns,
split across chains to balance); ACT ops are ~2x DVE cost on this HW.
"""

import numpy as np

H, MEM, S = 1024, 256, 272
B, T = 8, 4096
TOPK = 8
GAMMA, BETA, PTS = 0.92, 0.08, 0.4
PCH = 128          # timesteps per chunk
WARM = 128         # warmup steps per core window
SEG = T // 8       # output steps per core (512)
NCH = (WARM + SEG) // PCH   # chunks per chain (5)
WROWS = WARM + SEG          # x rows per chain per core (640)

_program_cache = {}


def _sigmoid(v):
    return 1.0 / (1.0 + np.exp(-v.astype(np.float64)))


TAPE_ACT = (0, 1, 2, 3, 4)  # chains whose tape-write runs on ACT (rest DVE)


def _build_program(loop_reps: int = 1, diag: tuple = ()):
    import concourse.bacc as bacc
    import concourse.mybir as mybir
    from concourse.tile import TileContext
    from concourse.masks import make_identity

    f32 = mybir.dt.float32
    Alu = mybir.AluOpType
    Act = mybir.ActivationFunctionType

    DG = set(diag)  # timing-only diagnostics (wrong math): 'no_pieces',
    # 'no_reduce', 'u_dve', 'no_tmp', 'no_d', 'no_tape'
    from concourse._compat import get_trn_type
    nc = bacc.Bacc(get_trn_type() or "TRN2", target_bir_lowering=False, debug=False)
    xb = nc.declare_dram_parameter("xb", [B * WROWS, H], f32, isOutput=False)
    basis_m = nc.declare_dram_parameter("basis_m", [128, 8 * 256], f32, isOutput=False)
    basis_y = nc.declare_dram_parameter("basis_y", [128, 2 * H], f32, isOutput=False)
    v0d = nc.declare_dram_parameter("v0", [128, 4], f32, isOutput=False)
    scal = nc.declare_dram_parameter("scal", [128, 3], f32, isOutput=False)  # [ones, gp, one]
    yb = nc.declare_dram_parameter("yb", [B * SEG, H], f32, isOutput=True)

    with TileContext(nc) as tc:
        with (
            tc.tile_pool(name="const", bufs=1) as cpool,
            tc.tile_pool(name="xio", bufs=3) as xpool,
            tc.tile_pool(name="work", bufs=2) as wpool,
            tc.tile_pool(name="scan", bufs=4) as spool,
            tc.tile_pool(name="ps_t", bufs=2, space="PSUM") as ps_t,
            tc.tile_pool(name="ps_m", bufs=2, space="PSUM") as ps_m,
            tc.tile_pool(name="ps_y", bufs=2, space="PSUM") as ps_y,
            tc.tile_pool(name="ps_r", bufs=2, space="PSUM") as ps_r,
        ):
            # ---- constants ----
            bm_sb = cpool.tile([128, 8 * 256], f32, tag="bm")
            nc.sync.dma_start(bm_sb, basis_m[:])
            by_sb = cpool.tile([128, 2 * H], f32, tag="by")
            nc.sync.dma_start(by_sb, basis_y[:])
            v0_sb = cpool.tile([128, 4], f32, tag="v0")
            nc.sync.dma_start(v0_sb, v0d[:])
            sc_sb = cpool.tile([128, 3], f32, tag="sc")
            nc.sync.dma_start(sc_sb, scal[:])
            ident = cpool.tile([128, 128], f32, tag="ident")
            make_identity(nc, ident[:])
            ones_mat = cpool.tile([128, 128], f32, tag="ones_mat")
            nc.vector.memset(ones_mat[:], 1.0)

            gp_ap = sc_sb[:, 1:2]

            # single-dep warmups so no first consumer combines two waits
            # (shares the "m" tag so PSUM stays within 8 banks)
            warm_acc = ps_m.tile([128, 1], f32, tag="m")

            def pe_touch(sb_slice):
                nc.tensor.matmul(
                    warm_acc[:], sb_slice, ident[:, 0:1], start=True, stop=True
                )

            pe_touch(ident[:, 0:128])
            pe_touch(bm_sb[:, 0:128])
            pe_touch(by_sb[:, 0:128])
            wsc = spool.tile([128, 4], f32, tag="wsc")
            nc.vector.tensor_copy(wsc[:], v0_sb[:])
            wsc2 = spool.tile([128, 3], f32, tag="wsc2")
            nc.vector.tensor_copy(wsc2[:], sc_sb[:])
            wsc3 = spool.tile([128, 3], f32, tag="wsc3")
            nc.scalar.copy(wsc3[:], sc_sb[:])
            wsc4 = spool.tile([128, 4], f32, tag="wsc4")
            nc.gpsimd.tensor_copy(wsc4[:], v0_sb[:])

            def prep_pieces(k, b, st):
                """Prep chunk k of chain b: x DMA, transpose, m-matmul,
                top-8 inject, injC. Returns list of emission pieces."""
                r0 = b * WROWS + k * PCH
                pieces = []

                def p_dma():
                    # x persists through this chunk's y-block (ident-matmul
                    # addend), so per-chain tag with 2 rotating buffers
                    st["x"] = xpool.tile(
                        [128, H], f32, tag=f"x{b}", bufs=2, name=f"x{k}_{b}"
                    )
                    nc.sync.dma_start(st["x"], xb[r0 : r0 + PCH, :])
                    st["xT"] = xpool.tile([128, H], f32, tag="xT", name=f"xT{k}_{b}")
                    # injC: step t's complex inject at cols 4t..4t+4 =
                    # (re_lo, re_hi, 0, 0) — contiguous per-step reads.
                    st["injC"] = wpool.tile(
                        [128, 512], f32, tag=f"injC{b}", name=f"injC{k}_{b}"
                    )
                    nc.gpsimd.memset(st["injC"][:], 0.0)

                pieces.append(p_dma)
                for hi in range(8):
                    def p_tr(hi=hi):
                        tps = ps_t.tile([128, 128], f32, tag="tps")
                        nc.tensor.transpose(
                            tps, st["x"][:, hi * 128 : (hi + 1) * 128], ident[:]
                        )
                        nc.scalar.copy(st["xT"][:, hi * 128 : (hi + 1) * 128], tps[:])

                    pieces.append(p_tr)

                def p_mm():
                    st["m"] = ps_m.tile([128, 256], f32, tag="m", name=f"m{k}_{b}")
                    for hi in range(8):
                        nc.tensor.matmul(
                            st["m"][:],
                            st["xT"][:, hi * 128 : (hi + 1) * 128],
                            bm_sb[:, hi * 256 : (hi + 1) * 256],
                            start=(hi == 0),
                            stop=(hi == 7),
                        )

                pieces.append(p_mm)

                def p_inj():
                    mag = wpool.tile([128, 256], f32, tag="mag")
                    nc.scalar.activation(mag[:], st["m"][:], Act.Abs)
                    mx8 = wpool.tile([128, 8], f32, tag="mx8")
                    nc.vector.max(mx8[:], mag[:])
                    inj = wpool.tile([128, 256], f32, tag="inj")
                    nc.vector.scalar_tensor_tensor(
                        out=inj[:], in0=mag[:], scalar=mx8[:, 7:8], in1=st["m"][:],
                        op0=Alu.is_ge, op1=Alu.mult,
                    )
                    st["inj"] = inj

                pieces.append(p_inj)
                for q in range(2):
                    def p_injC(q=q):
                        tps = ps_t.tile([128, 128], f32, tag="tps")
                        nc.tensor.transpose(
                            tps, st["inj"][:, q * 128 : (q + 1) * 128], ident[:]
                        )
                        nc.scalar.copy(st["injC"][:, q : 512 : 4], tps[:])

                    pieces.append(p_injC)
                return pieces

            def y_pieces(k, b, Z, x_sb):
                """y block for chunk k (k>=1) of chain b: y[t,h] = x[t,h] +
                sum_s Re(tape)[s,t]*WyT[s,h], via 3 accumulating matmuls per
                h-half (x re-used from prep); ACT copies to one full-width
                SBUF tile; ONE row-contiguous DMA out on the ACT ring."""
                yr0 = b * SEG + (k - 1) * PCH
                pieces = []
                st = {}

                def p_zre():
                    # de-stride Re(tape) once per chunk: strided PE A-operands
                    # are ~4x slower, the two ACT copies are cheaper
                    zre = wpool.tile([128, 256], f32, tag="zre")
                    nc.scalar.copy(zre[:, 0:128], Z[:, 0:512:4])
                    nc.scalar.copy(zre[:, 128:256], Z[:, 1:512:4])
                    st["zre"] = zre
                    st["ysb"] = xpool.tile([128, H], f32, tag="ysb", name="ysb")

                pieces.append(p_zre)
                for hh in range(2):
                    def p_mm(hh=hh):
                        y_ps = ps_y.tile([128, 512], f32, tag="yps")
                        nc.tensor.matmul(
                            y_ps[:], st["zre"][:, 0:128],
                            by_sb[:, hh * 512 : (hh + 1) * 512],
                            start=True, stop=False,
                        )
                        nc.tensor.matmul(
                            y_ps[:], st["zre"][:, 128:256],
                            by_sb[:, H + hh * 512 : H + (hh + 1) * 512],
                            start=False, stop=False,
                        )
                        nc.tensor.matmul(
                            y_ps[:], ident[:],
                            x_sb[:, hh * 512 : (hh + 1) * 512],
                            start=False, stop=True,
                        )
                        st[f"yps_{hh}"] = y_ps

                    def p_cp(hh=hh):
                        nc.scalar.copy(
                            st["ysb"][:, hh * 512 : (hh + 1) * 512],
                            st[f"yps_{hh}"][:],
                        )

                    pieces += [p_mm, p_cp]

                def p_dma_out():
                    nc.scalar.dma_start(yb[yr0 : yr0 + PCH, :], st["ysb"][:])

                pieces.append(p_dma_out)
                return pieces

            from contextlib import nullcontext
            if "no_prep" in DG:
                DG.add("dummy_inj")
            if "no_pieces" in DG:
                DG.update(("dummy_inj", "no_prep", "no_y"))
            dummy_injC = None
            if "dummy_inj" in DG:
                dummy_injC = []
                for b in range(B):
                    dc = cpool.tile([128, 512], f32, tag=f"dinjC{b}")
                    nc.gpsimd.memset(dc[:], 0.0)
                    dummy_injC.append(dc)
            rep_ctx = tc.For_i(0, loop_reps, 1) if loop_reps > 1 else nullcontext()
            with rep_ctx:
                # prep chunk 0 for all chains (startup bubble)
                cur = [dict() for _ in range(B)]
                if "no_prep" not in DG:
                    for b in range(B):
                        for p in prep_pieces(0, b, cur[b]):
                            p()

                Z_prev = [None] * B
                prev_x = [None] * B
                for k in range(NCH):
                    # Z: per-chunk tape state (and y-source); step t's complex
                    # tape at cols 4t..4t+4 = (re_lo, re_hi, im_lo, im_hi)
                    Z = [
                        wpool.tile([128, 512], f32, tag=f"Z{b}", name=f"Z{k}_{b}")
                        for b in range(B)
                    ]

                    # schedule interleaved pieces: per chain, y(k-1) first,
                    # then prep(k+1) — prep's x-DMA recycles the x buffer
                    # that y(k-1)'s ident-matmul still reads, so emission
                    # order must put the reads first
                    sched = {}
                    nxt = [dict() for _ in range(B)]
                    for b in range(B):
                        s0 = 2 + 15 * b
                        if k >= 2 and "no_y" not in DG:
                            yp = y_pieces(k - 1, b, Z_prev[b], prev_x[b])
                            for i, p in enumerate(yp):
                                sched.setdefault(s0 + i // 2, []).append(p)
                            s0 += 4
                        if k + 1 < NCH and "no_prep" not in DG:
                            for i, p in enumerate(prep_pieces(k + 1, b, nxt[b])):
                                sched.setdefault(s0 + i // 2, []).append(p)

                    if "dummy_inj" in DG:
                        injC = dummy_injC
                    else:
                        injC = [cur[b]["injC"] for b in range(B)]
                    for t in range(PCH):
                        def tape_at(b, tt):
                            if tt < 0:
                                if k == 0:
                                    return v0_sb[:]
                                return Z_prev[b][:, 4 * (PCH + tt) : 4 * (PCH + tt) + 4]
                            return Z[b][:, 4 * tt : 4 * tt + 4]

                        sqs = spool.tile([128, 8], f32, tag="sq")
                        tmp_t = [None] * B
                        u_t = [None] * B
                        for b in range(B):
                            if "no_tmp" in DG:
                                tmp_t[b] = injC[b][:, 4 * t : 4 * t + 4]
                                continue
                            tmp = spool.tile([128, 4], f32, tag=f"tmp_{b}")
                            nc.vector.scalar_tensor_tensor(
                                out=tmp[:], in0=tape_at(b, t - 2), scalar=gp_ap,
                                in1=injC[b][:, 4 * t : 4 * t + 4],
                                op0=Alu.mult, op1=Alu.add,
                            )
                            tmp_t[b] = tmp
                        for b in range(B):
                            u = spool.tile([128, 4], f32, tag=f"u_{b}")
                            if "u_dve" in DG:
                                nc.vector.tensor_tensor(
                                    out=u[:], in0=tmp_t[b][:], in1=tape_at(b, t - 1),
                                    op=Alu.add,
                                )
                            else:
                                nc.gpsimd.tensor_tensor(
                                    out=u[:], in0=tmp_t[b][:], in1=tape_at(b, t - 1),
                                    op=Alu.add,
                                )
                            u_t[b] = u
                        if "no_d" not in DG:
                            for b in range(B):
                                d = spool.tile([128, 4], f32, tag=f"d_{b}")
                                nc.vector.scalar_tensor_tensor(
                                    out=d[:], in0=u_t[b][:], scalar=1.0, in1=u_t[b][:],
                                    op0=Alu.mult, op1=Alu.mult,
                                    accum_out=sqs[:, b : b + 1],
                                )
                        if "no_reduce" not in DG and "no_d" not in DG:
                            # one PE matmul reduces+broadcasts all chains at once
                            sqr = ps_r.tile([128, 8], f32, tag="ps_b")
                            nc.tensor.matmul(
                                sqr[:], ones_mat[:], sqs[:], start=True, stop=True
                            )
                            # one batched ACT rsqrt for all chains
                            r8 = spool.tile([128, 8], f32, tag="r8")
                            nc.scalar.activation(r8[:], sqr[:], Act.Abs_reciprocal_sqrt)
                            r8ap = r8
                        else:
                            r8ap = None
                        if "no_tape" not in DG:
                            for b in range(B):
                                dst = Z[b][:, 4 * t : 4 * t + 4]
                                scl = (
                                    r8ap[:, b : b + 1] if r8ap is not None else gp_ap
                                )
                                if b in TAPE_ACT:
                                    nc.scalar.activation(
                                        dst, u_t[b][:], Act.Copy, scale=scl
                                    )
                                else:
                                    nc.vector.tensor_scalar_mul(
                                        dst, u_t[b][:], scl
                                    )

                        for p in sched.get(t, ()):
                            p()

                    Z_prev = Z
                    prev_x = [cur[b].get("x") for b in range(B)]
                    cur = nxt

                # tail: y for the last chunk
                if "no_y" not in DG:
                    for b in range(B):
                        for p in y_pieces(NCH - 1, b, Z_prev[b], prev_x[b]):
                            p()

    nc.compile()
    return nc


def _host_pack(inputs):
    """Fold all small parameters host-side; returns shared constant arrays."""
    basis = np.asarray(inputs["basis"], np.float32)
    alpha = float(np.asarray(inputs["alpha"]))
    w_r = np.asarray(inputs["w_r"], np.float32)
    bg = _sigmoid(np.asarray(inputs["breadth_gate"], np.float32))

    g = _sigmoid(w_r)
    assert np.all(g[:MEM] == g[0]), "vector w_r gate not supported by fast path"
    gp = float(g[0]) / GAMMA

    Wm = (basis[:, :MEM] * (BETA / GAMMA)).astype(np.float32)  # [H, 256]
    Wy = (basis[:, :MEM] * (alpha * bg[None, :MEM])).astype(np.float32)

    basis_m = np.concatenate(
        [Wm[hi * 128 : (hi + 1) * 128, :] for hi in range(8)], axis=1
    ).astype(np.float32)  # [128, 2048]
    WyT = np.ascontiguousarray(Wy.T)  # [256, 1024]
    basis_y = np.concatenate([WyT[0:128, :], WyT[128:256, :]], axis=1).astype(
        np.float32
    )  # [128, 2048]

    t0c = (
        np.asarray(inputs["tape_init_re"], np.float32)
        + 1j * np.asarray(inputs["tape_init_im"], np.float32)
    )[:MEM].astype(np.complex64)
    nrm = np.float32(np.sqrt(max(float((np.abs(t0c) ** 2).sum(dtype=np.float32)), 1e-16)))
    v0c = (t0c / nrm).astype(np.complex64)
    v0 = np.stack(
        [v0c.real[:128], v0c.real[128:], v0c.imag[:128], v0c.imag[128:]], axis=1
    ).astype(np.float32)  # [128, 4]

    scal = np.empty((128, 3), np.float32)
    scal[:, 0] = 1.0
    scal[:, 1] = gp
    scal[:, 2] = 1.0
    return basis_m, basis_y, v0, scal


def _core_x_window(x, c):
    """[B*WROWS, H] x-window for core c: per chain, WARM warmup rows (zeros
    for c=0) then SEG output rows."""
    t0 = c * SEG
    win = np.zeros((B, WROWS, H), np.float32)
    if c > 0:
        win[:, :WARM] = x[:, t0 - WARM : t0]
    win[:, WARM:] = x[:, t0 : t0 + SEG]
    return np.ascontiguousarray(win.reshape(B * WROWS, H))


def _fast_path_ok(inputs):
    z = lambda k: np.all(np.asarray(inputs[k]) == 0)
    g = _sigmoid(np.asarray(inputs["w_r"], np.float32))
    return (
        z("torque_rotation")
        and z("epsilon_scale")
        and z("epsilon_diag")
        and z("pred_scale")
        and z("pred_diag")
        and bool(np.all(g[:MEM] == g[0]))
    )


def _numpy_fallback(inputs):
    """General-case reference implementation (host). Only used if the inputs
    violate the fast-path structure (never the case for this problem's
    generator); keeps kernel() total."""
    import jax

    with jax.default_device(jax.devices("cpu")[0]):
        import jax.numpy as jnp
        from jax import lax

        x = jnp.asarray(inputs["x"])
        basis = jnp.asarray(inputs["basis"])
        active = jnp.arange(S) < MEM
        amf = active.astype(jnp.float32)
        eta = jax.nn.softplus(jnp.asarray(inputs["eta_raw"]))
        eps = (jnp.asarray(inputs["epsilon_factor"]) * jnp.asarray(inputs["epsilon_scale"])) @ jnp.asarray(
            inputs["epsilon_factor"]).T + jnp.diag(jnp.asarray(inputs["epsilon_diag"]))
        wp = (jnp.asarray(inputs["pred_factor"]) * jnp.asarray(inputs["pred_scale"])) @ jnp.asarray(
            inputs["pred_factor"]).T + jnp.diag(jnp.asarray(inputs["pred_diag"]))
        eps_c = eps.astype(jnp.complex64)
        wp_c = wp.astype(jnp.complex64)
        rot = jnp.exp(1j * jnp.asarray(inputs["torque_rotation"]).astype(jnp.complex64))
        wr_gate = jax.nn.sigmoid(jnp.asarray(inputs["w_r"]))
        bg = jax.nn.sigmoid(jnp.asarray(inputs["breadth_gate"]))
        alpha = jnp.asarray(inputs["alpha"])

        def renorm(tape):
            masked = tape * amf
            nrm = jnp.sqrt(jnp.maximum((jnp.abs(masked) ** 2).sum(-1, keepdims=True), 1e-16))
            return masked / nrm

        tape0 = (jnp.asarray(inputs["tape_init_re"]) + 1j * jnp.asarray(inputs["tape_init_im"])) * amf
        tape0 = renorm(jnp.broadcast_to(tape0, (B, S)))

        def step(carry, x_t):
            tape, prev = carry
            m = jnp.einsum("hs,bh->bs", basis, x_t)
            mag = jnp.abs(m) * amf
            kth = lax.top_k(mag, TOPK)[0][:, -1:]
            injv = jnp.where((mag >= kth) & active, m, 0.0).astype(jnp.complex64)
            rotated = tape * rot
            drive = jnp.einsum("st,bt->bs", eps_c, rotated)
            pred = jnp.einsum("st,bt->bs", wp_c, rotated)
            new = (GAMMA * rotated + eta * drive + BETA * injv + PTS * 1j * pred + wr_gate * prev)
            new = renorm(new)
            y = x_t + alpha * jnp.einsum("hs,bs->bh", basis, bg * new.real)
            return (new, tape), y

        (_, _), ys = lax.scan(step, (tape0, tape0), jnp.swapaxes(x, 0, 1))
        return np.asarray(jnp.swapaxes(ys, 0, 1))


def _timing_build(loop_reps: int = 1):
    """Builder used by kernel() and test.py's repetition timer."""
    return _build_program(loop_reps=loop_reps)


def kernel(_want_trace: bool = False, **inputs) -> np.ndarray:
    from concourse.bass_utils import run_bass_kernel_spmd

    x = np.ascontiguousarray(np.asarray(inputs["x"], np.float32))
    assert x.shape == (B, T, H)

    if not _fast_path_ok(inputs):
        return _numpy_fallback(inputs)

    basis_m, basis_y, v0, scal = _host_pack(inputs)

    if "prog" not in _program_cache:
        _program_cache["prog"] = _timing_build()
    nc = _program_cache["prog"]

    core_ids = list(range(8))
    in_maps = [
        {
            "xb": _core_x_window(x, c),
            "basis_m": basis_m,
            "basis_y": basis_y,
            "v0": v0,
            "scal": scal,
        }
        for c in core_ids
    ]
    res = run_bass_kernel_spmd(nc, in_maps, core_ids, trace=_want_trace)
    out = np.empty((B, T, H), np.float32)
    for c in core_ids:
        ybc = res.results[c]["yb"].reshape(B, SEG, H)
        out[:, c * SEG : (c + 1) * SEG] = ybc
    if _want_trace:
        kernel._last_results = res
    return out


# revision 27
# speedup vs baseline: 1.2396x; 1.2396x over previous
"""Trainium2 Bass kernel for nn_MemoryEngineLayer (scatter_memory).

Contract: kernel(**inputs) takes FULL unsharded inputs (as produced by
setup_inputs()) and returns the FULL [B, T, H] output.

Sharding: sequence-parallel with warmup. Core c handles the T-window
[c*512, (c+1)*512) for ALL B=8 batch sequences. Each window is preceded by
W=128 warmup steps (real x for c>0; zeros for c=0 — with zero input the
recurrence state (v0, v0) is an exact fixed point: renorm((g+gp)*v0) = v0,
so core 0's math is exact). The top-k injection is state-independent, and
the gamma=0.92 contraction + renorm forgets the initial state: warmup of
128 steps leaves ~5e-3 relative tape error at the window start, decaying to
~1e-5 within 256 steps; the y-output is dominated by the x passthrough
(tape term is ~0.4% of ||y||), so the global rel err stays ~1e-5.

Why: the scan's serial chain (DVE stt -> PE partition-reduce -> ACT rsqrt ->
next step's DVE) is latency-bound at ~2.2us/step when one chain runs alone.
With all 8 batch chains interleaved per core, each engine processes 8
independent steps' work back-to-back and the cross-engine latency is hidden;
throughput becomes issue-bound (~0.3us/step).

Math per chain (normalized-tape form; equivalent to the reference):
  m'_t   = x_t @ (beta/gamma * basis[:, :256])           # [256], top-8 inject
  inj'_t = where(|m'_t| >= kth8(|m'_t|), m'_t, 0)
  u_t    = tape_{t-1} + (g/gamma) tape_{t-2} + inj'_t     (complex 256)
  r_t    = rsqrt(||u_t||^2)
  tape_t = r_t u_t
  y_t    = x_t + Re(tape_t) @ (alpha * bg * basis[:, :256]).T
Carrying the normalized tape as state removes the r_{t-1}/gr scalar chains
from the vector recurrence entirely: only tape values cross steps.

On-chip layout: complex state as [128 partitions, 4 cols] = (re_lo, re_hi,
im_lo, im_hi) with slot s = q*128 + p; the per-step state lives at cols
4t..4t+4 of the per-chunk Z buffer (contiguous, also the y-matmul source).
Per macro-step, all 8 chains share ONE PE ones-matmul partition-reduce
([128,8]) and ONE batched ACT rsqrt producing r for all chains at once.
Engine budget per chain-step: DVE stt tmp (~123ns) + DVE accum d (~111ns)
+ Pool tensor_tensor u (~138ns) + tape write (ACT ~281ns / DVE ~144ns,
split across chains to balance); ACT ops are ~2x DVE cost on this HW.
"""

import numpy as np

H, MEM, S = 1024, 256, 272
B, T = 8, 4096
TOPK = 8
GAMMA, BETA, PTS = 0.92, 0.08, 0.4
PCH = 128          # timesteps per full chunk
WARM = 64          # warmup steps per core window (chunk 0 is short)
SEG = T // 8       # output steps per core (512)
NCH = 5            # chunks per chain: one short warmup + 4 output chunks
CHLEN = (WARM, PCH, PCH, PCH, PCH)
XOFF = (0, WARM, WARM + PCH, WARM + 2 * PCH, WARM + 3 * PCH)
WROWS = WARM + SEG          # x rows per chain per core (576)

_program_cache = {}


def _sigmoid(v):
    return 1.0 / (1.0 + np.exp(-v.astype(np.float64)))


TAPE_ACT = (0, 1, 2, 3, 4)  # chains whose tape-write runs on ACT (rest DVE)


def _build_program(loop_reps: int = 1, diag: tuple = ()):
    import concourse.bacc as bacc
    import concourse.mybir as mybir
    from concourse.tile import TileContext
    from concourse.masks import make_identity

    f32 = mybir.dt.float32
    Alu = mybir.AluOpType
    Act = mybir.ActivationFunctionType

    DG = set(diag)  # timing-only diagnostics (wrong math): 'no_pieces',
    # 'no_reduce', 'u_dve', 'no_tmp', 'no_d', 'no_tape'
    from concourse._compat import get_trn_type
    nc = bacc.Bacc(get_trn_type() or "TRN2", target_bir_lowering=False, debug=False)
    xb = nc.declare_dram_parameter("xb", [B * WROWS, H], f32, isOutput=False)
    basis_m = nc.declare_dram_parameter("basis_m", [128, 8 * 256], f32, isOutput=False)
    basis_y = nc.declare_dram_parameter("basis_y", [128, 2 * H], f32, isOutput=False)
    v0d = nc.declare_dram_parameter("v0", [128, 4], f32, isOutput=False)
    scal = nc.declare_dram_parameter("scal", [128, 3], f32, isOutput=False)  # [ones, gp, one]
    yb = nc.declare_dram_parameter("yb", [B * SEG, H], f32, isOutput=True)

    with TileContext(nc) as tc:
        with (
            tc.tile_pool(name="const", bufs=1) as cpool,
            tc.tile_pool(name="xio", bufs=3) as xpool,
            tc.tile_pool(name="work", bufs=2) as wpool,
            tc.tile_pool(name="scan", bufs=4) as spool,
            tc.tile_pool(name="ps_t", bufs=2, space="PSUM") as ps_t,
            tc.tile_pool(name="ps_m", bufs=2, space="PSUM") as ps_m,
            tc.tile_pool(name="ps_y", bufs=2, space="PSUM") as ps_y,
            tc.tile_pool(name="ps_r", bufs=2, space="PSUM") as ps_r,
        ):
            # ---- constants ----
            bm_sb = cpool.tile([128, 8 * 256], f32, tag="bm")
            nc.sync.dma_start(bm_sb, basis_m[:])
            by_sb = cpool.tile([128, 2 * H], f32, tag="by")
            nc.sync.dma_start(by_sb, basis_y[:])
            v0_sb = cpool.tile([128, 4], f32, tag="v0")
            nc.sync.dma_start(v0_sb, v0d[:])
            sc_sb = cpool.tile([128, 3], f32, tag="sc")
            nc.sync.dma_start(sc_sb, scal[:])
            ident = cpool.tile([128, 128], f32, tag="ident")
            make_identity(nc, ident[:])
            ones_mat = cpool.tile([128, 128], f32, tag="ones_mat")
            nc.vector.memset(ones_mat[:], 1.0)

            gp_ap = sc_sb[:, 1:2]

            # single-dep warmups so no first consumer combines two waits
            # (shares the "m" tag so PSUM stays within 8 banks)
            warm_acc = ps_m.tile([128, 1], f32, tag="m")

            def pe_touch(sb_slice):
                nc.tensor.matmul(
                    warm_acc[:], sb_slice, ident[:, 0:1], start=True, stop=True
                )

            pe_touch(ident[:, 0:128])
            pe_touch(bm_sb[:, 0:128])
            pe_touch(by_sb[:, 0:128])
            wsc = spool.tile([128, 4], f32, tag="wsc")
            nc.vector.tensor_copy(wsc[:], v0_sb[:])
            wsc2 = spool.tile([128, 3], f32, tag="wsc2")
            nc.vector.tensor_copy(wsc2[:], sc_sb[:])
            wsc3 = spool.tile([128, 3], f32, tag="wsc3")
            nc.scalar.copy(wsc3[:], sc_sb[:])
            wsc4 = spool.tile([128, 4], f32, tag="wsc4")
            nc.gpsimd.tensor_copy(wsc4[:], v0_sb[:])

            def prep_pieces(k, b, st):
                """Prep chunk k of chain b: x DMA, transpose, m-matmul,
                top-8 inject, injC. Returns list of emission pieces."""
                r0 = b * WROWS + XOFF[k]
                nrow = min(PCH, WROWS - XOFF[k])
                pieces = []

                def p_dma():
                    # x persists through this chunk's y-block (ident-matmul
                    # addend), so per-chain tag with 2 rotating buffers
                    st["x"] = xpool.tile(
                        [128, H], f32, tag=f"x{b}", bufs=2, name=f"x{k}_{b}"
                    )
                    nc.sync.dma_start(st["x"], xb[r0 : r0 + nrow, :])
                    st["xT"] = xpool.tile([128, H], f32, tag="xT", name=f"xT{k}_{b}")
                    # injC: step t's complex inject at cols 4t..4t+4 =
                    # (re_lo, re_hi, 0, 0) — contiguous per-step reads.
                    st["injC"] = wpool.tile(
                        [128, 512], f32, tag=f"injC{b}", name=f"injC{k}_{b}"
                    )
                    nc.gpsimd.memset(st["injC"][:], 0.0)

                pieces.append(p_dma)
                for hi in range(8):
                    def p_tr(hi=hi):
                        tps = ps_t.tile([128, 128], f32, tag="tps")
                        nc.tensor.transpose(
                            tps, st["x"][:, hi * 128 : (hi + 1) * 128], ident[:]
                        )
                        nc.scalar.copy(st["xT"][:, hi * 128 : (hi + 1) * 128], tps[:])

                    pieces.append(p_tr)

                def p_mm():
                    st["m"] = ps_m.tile([128, 256], f32, tag="m", name=f"m{k}_{b}")
                    for hi in range(8):
                        nc.tensor.matmul(
                            st["m"][:],
                            st["xT"][:, hi * 128 : (hi + 1) * 128],
                            bm_sb[:, hi * 256 : (hi + 1) * 256],
                            start=(hi == 0),
                            stop=(hi == 7),
                        )

                pieces.append(p_mm)

                def p_inj():
                    mag = wpool.tile([128, 256], f32, tag="mag")
                    nc.scalar.activation(mag[:], st["m"][:], Act.Abs)
                    mx8 = wpool.tile([128, 8], f32, tag="mx8")
                    nc.vector.max(mx8[:], mag[:])
                    inj = wpool.tile([128, 256], f32, tag="inj")
                    nc.vector.scalar_tensor_tensor(
                        out=inj[:], in0=mag[:], scalar=mx8[:, 7:8], in1=st["m"][:],
                        op0=Alu.is_ge, op1=Alu.mult,
                    )
                    st["inj"] = inj

                pieces.append(p_inj)
                for q in range(2):
                    def p_injC(q=q):
                        tps = ps_t.tile([128, 128], f32, tag="tps")
                        nc.tensor.transpose(
                            tps, st["inj"][:, q * 128 : (q + 1) * 128], ident[:]
                        )
                        nc.scalar.copy(st["injC"][:, q : 512 : 4], tps[:])

                    pieces.append(p_injC)
                return pieces

            def y_pieces(k, b, Z, x_sb):
                """y block for chunk k (k>=1) of chain b: y[t,h] = x[t,h] +
                sum_s Re(tape)[s,t]*WyT[s,h], via 3 accumulating matmuls per
                h-half (x re-used from prep); ACT copies to one full-width
                SBUF tile; ONE row-contiguous DMA out on the ACT ring."""
                yr0 = b * SEG + (k - 1) * PCH
                pieces = []
                st = {}

                def p_zre():
                    # de-stride Re(tape) once per chunk: strided PE A-operands
                    # are ~4x slower, the two ACT copies are cheaper
                    zre = wpool.tile([128, 256], f32, tag="zre")
                    nc.scalar.copy(zre[:, 0:128], Z[:, 0:512:4])
                    nc.scalar.copy(zre[:, 128:256], Z[:, 1:512:4])
                    st["zre"] = zre
                    st["ysb"] = xpool.tile([128, H], f32, tag="ysb", name="ysb")

                pieces.append(p_zre)
                for hh in range(2):
                    def p_mm(hh=hh):
                        y_ps = ps_y.tile([128, 512], f32, tag="yps")
                        nc.tensor.matmul(
                            y_ps[:], st["zre"][:, 0:128],
                            by_sb[:, hh * 512 : (hh + 1) * 512],
                            start=True, stop=False,
                        )
                        nc.tensor.matmul(
                            y_ps[:], st["zre"][:, 128:256],
                            by_sb[:, H + hh * 512 : H + (hh + 1) * 512],
                            start=False, stop=False,
                        )
                        nc.tensor.matmul(
                            y_ps[:], ident[:],
                            x_sb[:, hh * 512 : (hh + 1) * 512],
                            start=False, stop=True,
                        )
                        st[f"yps_{hh}"] = y_ps

                    def p_cp(hh=hh):
                        nc.scalar.copy(
                            st["ysb"][:, hh * 512 : (hh + 1) * 512],
                            st[f"yps_{hh}"][:],
                        )

                    pieces += [p_mm, p_cp]

                def p_dma_out():
                    nc.scalar.dma_start(yb[yr0 : yr0 + PCH, :], st["ysb"][:])

                pieces.append(p_dma_out)
                return pieces

            from contextlib import nullcontext
            if "no_prep" in DG:
                DG.add("dummy_inj")
            if "no_pieces" in DG:
                DG.update(("dummy_inj", "no_prep", "no_y"))
            dummy_injC = None
            if "dummy_inj" in DG:
                dummy_injC = []
                for b in range(B):
                    dc = cpool.tile([128, 512], f32, tag=f"dinjC{b}")
                    nc.gpsimd.memset(dc[:], 0.0)
                    dummy_injC.append(dc)
            rep_ctx = tc.For_i(0, loop_reps, 1) if loop_reps > 1 else nullcontext()
            with rep_ctx:
                # prep chunk 0 for all chains (startup bubble)
                cur = [dict() for _ in range(B)]
                if "no_prep" not in DG:
                    for b in range(B):
                        for p in prep_pieces(0, b, cur[b]):
                            p()

                Z_prev = [None] * B
                prev_x = [None] * B
                for k in range(NCH):
                    # Z: per-chunk tape state (and y-source); step t's complex
                    # tape at cols 4t..4t+4 = (re_lo, re_hi, im_lo, im_hi)
                    Z = [
                        wpool.tile([128, 512], f32, tag=f"Z{b}", name=f"Z{k}_{b}")
                        for b in range(B)
                    ]

                    # schedule interleaved pieces: per chain, y(k-1) first,
                    # then prep(k+1) — prep's x-DMA recycles the x buffer
                    # that y(k-1)'s ident-matmul still reads, so emission
                    # order must put the reads first
                    sched = {}
                    nxt = [dict() for _ in range(B)]
                    # short warmup chunk has fewer slots: pack denser there
                    stride, pack = (15, 2) if CHLEN[k] == PCH else (7, 3)
                    for b in range(B):
                        s0 = 1 + stride * b
                        if k >= 2 and "no_y" not in DG:
                            yp = y_pieces(k - 1, b, Z_prev[b], prev_x[b])
                            for i, p in enumerate(yp):
                                sched.setdefault(s0 + i // pack, []).append(p)
                            s0 += (len(yp) + pack - 1) // pack
                        if k + 1 < NCH and "no_prep" not in DG:
                            for i, p in enumerate(prep_pieces(k + 1, b, nxt[b])):
                                sched.setdefault(s0 + i // pack, []).append(p)

                    if "dummy_inj" in DG:
                        injC = dummy_injC
                    else:
                        injC = [cur[b]["injC"] for b in range(B)]
                    Lk = CHLEN[k]
                    Lp = CHLEN[k - 1] if k > 0 else 0
                    for t in range(Lk):
                        def tape_at(b, tt):
                            if tt < 0:
                                if k == 0:
                                    return v0_sb[:]
                                return Z_prev[b][:, 4 * (Lp + tt) : 4 * (Lp + tt) + 4]
                            return Z[b][:, 4 * tt : 4 * tt + 4]

                        sqs = spool.tile([128, 8], f32, tag="sq")
                        tmp_t = [None] * B
                        u_t = [None] * B
                        for b in range(B):
                            if "no_tmp" in DG:
                                tmp_t[b] = injC[b][:, 4 * t : 4 * t + 4]
                                continue
                            tmp = spool.tile([128, 4], f32, tag=f"tmp_{b}")
                            nc.vector.scalar_tensor_tensor(
                                out=tmp[:], in0=tape_at(b, t - 2), scalar=gp_ap,
                                in1=injC[b][:, 4 * t : 4 * t + 4],
                                op0=Alu.mult, op1=Alu.add,
                            )
                            tmp_t[b] = tmp
                        for b in range(B):
                            u = spool.tile([128, 4], f32, tag=f"u_{b}")
                            if "u_dve" in DG:
                                nc.vector.tensor_tensor(
                                    out=u[:], in0=tmp_t[b][:], in1=tape_at(b, t - 1),
                                    op=Alu.add,
                                )
                            else:
                                nc.gpsimd.tensor_tensor(
                                    out=u[:], in0=tmp_t[b][:], in1=tape_at(b, t - 1),
                                    op=Alu.add,
                                )
                            u_t[b] = u
                        if "no_d" not in DG:
                            for b in range(B):
                                d = spool.tile([128, 4], f32, tag=f"d_{b}")
                                nc.vector.scalar_tensor_tensor(
                                    out=d[:], in0=u_t[b][:], scalar=1.0, in1=u_t[b][:],
                                    op0=Alu.mult, op1=Alu.mult,
                                    accum_out=sqs[:, b : b + 1],
                                )
                        if "no_reduce" not in DG and "no_d" not in DG:
                            # one PE matmul reduces+broadcasts all chains at once
                            sqr = ps_r.tile([128, 8], f32, tag="ps_b")
                            nc.tensor.matmul(
                                sqr[:], ones_mat[:], sqs[:], start=True, stop=True
                            )
                            # one batched ACT rsqrt for all chains
                            r8 = spool.tile([128, 8], f32, tag="r8")
                            nc.scalar.activation(r8[:], sqr[:], Act.Abs_reciprocal_sqrt)
                            r8ap = r8
                        else:
                            r8ap = None
                        if "no_tape" not in DG:
                            for b in range(B):
                                dst = Z[b][:, 4 * t : 4 * t + 4]
                                scl = (
                                    r8ap[:, b : b + 1] if r8ap is not None else gp_ap
                                )
                                if b in TAPE_ACT:
                                    nc.scalar.activation(
                                        dst, u_t[b][:], Act.Copy, scale=scl
                                    )
                                else:
                                    nc.vector.tensor_scalar_mul(
                                        dst, u_t[b][:], scl
                                    )

                        for p in sched.get(t, ()):
                            p()

                    Z_prev = Z
                    prev_x = [cur[b].get("x") for b in range(B)]
                    cur = nxt

                # tail: y for the last chunk
                if "no_y" not in DG:
                    for b in range(B):
                        for p in y_pieces(NCH - 1, b, Z_prev[b], prev_x[b]):
                            p()

    nc.compile()
    return nc


def _host_pack(inputs):
    """Fold all small parameters host-side; returns shared constant arrays."""
    basis = np.asarray(inputs["basis"], np.float32)
    alpha = float(np.asarray(inputs["alpha"]))
    w_r = np.asarray(inputs["w_r"], np.float32)
    bg = _sigmoid(np.asarray(inputs["breadth_gate"], np.float32))

    g = _sigmoid(w_r)
    assert np.all(g[:MEM] == g[0]), "vector w_r gate not supported by fast path"
    gp = float(g[0]) / GAMMA

    Wm = (basis[:, :MEM] * (BETA / GAMMA)).astype(np.float32)  # [H, 256]
    Wy = (basis[:, :MEM] * (alpha * bg[None, :MEM])).astype(np.float32)

    basis_m = np.concatenate(
        [Wm[hi * 128 : (hi + 1) * 128, :] for hi in range(8)], axis=1
    ).astype(np.float32)  # [128, 2048]
    WyT = np.ascontiguousarray(Wy.T)  # [256, 1024]
    basis_y = np.concatenate([WyT[0:128, :], WyT[128:256, :]], axis=1).astype(
        np.float32
    )  # [128, 2048]

    t0c = (
        np.asarray(inputs["tape_init_re"], np.float32)
        + 1j * np.asarray(inputs["tape_init_im"], np.float32)
    )[:MEM].astype(np.complex64)
    nrm = np.float32(np.sqrt(max(float((np.abs(t0c) ** 2).sum(dtype=np.float32)), 1e-16)))
    v0c = (t0c / nrm).astype(np.complex64)
    v0 = np.stack(
        [v0c.real[:128], v0c.real[128:], v0c.imag[:128], v0c.imag[128:]], axis=1
    ).astype(np.float32)  # [128, 4]

    scal = np.empty((128, 3), np.float32)
    scal[:, 0] = 1.0
    scal[:, 1] = gp
    scal[:, 2] = 1.0
    return basis_m, basis_y, v0, scal


def _core_x_window(x, c):
    """[B*WROWS, H] x-window for core c: per chain, WARM warmup rows (zeros
    for c=0) then SEG output rows."""
    t0 = c * SEG
    win = np.zeros((B, WROWS, H), np.float32)
    if c > 0:
        win[:, :WARM] = x[:, t0 - WARM : t0]
    win[:, WARM:] = x[:, t0 : t0 + SEG]
    return np.ascontiguousarray(win.reshape(B * WROWS, H))


def _fast_path_ok(inputs):
    z = lambda k: np.all(np.asarray(inputs[k]) == 0)
    g = _sigmoid(np.asarray(inputs["w_r"], np.float32))
    return (
        z("torque_rotation")
        and z("epsilon_scale")
        and z("epsilon_diag")
        and z("pred_scale")
        and z("pred_diag")
        and bool(np.all(g[:MEM] == g[0]))
    )


def _numpy_fallback(inputs):
    """General-case reference implementation (host). Only used if the inputs
    violate the fast-path structure (never the case for this problem's
    generator); keeps kernel() total."""
    import jax

    with jax.default_device(jax.devices("cpu")[0]):
        import jax.numpy as jnp
        from jax import lax

        x = jnp.asarray(inputs["x"])
        basis = jnp.asarray(inputs["basis"])
        active = jnp.arange(S) < MEM
        amf = active.astype(jnp.float32)
        eta = jax.nn.softplus(jnp.asarray(inputs["eta_raw"]))
        eps = (jnp.asarray(inputs["epsilon_factor"]) * jnp.asarray(inputs["epsilon_scale"])) @ jnp.asarray(
            inputs["epsilon_factor"]).T + jnp.diag(jnp.asarray(inputs["epsilon_diag"]))
        wp = (jnp.asarray(inputs["pred_factor"]) * jnp.asarray(inputs["pred_scale"])) @ jnp.asarray(
            inputs["pred_factor"]).T + jnp.diag(jnp.asarray(inputs["pred_diag"]))
        eps_c = eps.astype(jnp.complex64)
        wp_c = wp.astype(jnp.complex64)
        rot = jnp.exp(1j * jnp.asarray(inputs["torque_rotation"]).astype(jnp.complex64))
        wr_gate = jax.nn.sigmoid(jnp.asarray(inputs["w_r"]))
        bg = jax.nn.sigmoid(jnp.asarray(inputs["breadth_gate"]))
        alpha = jnp.asarray(inputs["alpha"])

        def renorm(tape):
            masked = tape * amf
            nrm = jnp.sqrt(jnp.maximum((jnp.abs(masked) ** 2).sum(-1, keepdims=True), 1e-16))
            return masked / nrm

        tape0 = (jnp.asarray(inputs["tape_init_re"]) + 1j * jnp.asarray(inputs["tape_init_im"])) * amf
        tape0 = renorm(jnp.broadcast_to(tape0, (B, S)))

        def step(carry, x_t):
            tape, prev = carry
            m = jnp.einsum("hs,bh->bs", basis, x_t)
            mag = jnp.abs(m) * amf
            kth = lax.top_k(mag, TOPK)[0][:, -1:]
            injv = jnp.where((mag >= kth) & active, m, 0.0).astype(jnp.complex64)
            rotated = tape * rot
            drive = jnp.einsum("st,bt->bs", eps_c, rotated)
            pred = jnp.einsum("st,bt->bs", wp_c, rotated)
            new = (GAMMA * rotated + eta * drive + BETA * injv + PTS * 1j * pred + wr_gate * prev)
            new = renorm(new)
            y = x_t + alpha * jnp.einsum("hs,bs->bh", basis, bg * new.real)
            return (new, tape), y

        (_, _), ys = lax.scan(step, (tape0, tape0), jnp.swapaxes(x, 0, 1))
        return np.asarray(jnp.swapaxes(ys, 0, 1))


def _timing_build(loop_reps: int = 1):
    """Builder used by kernel() and test.py's repetition timer."""
    return _build_program(loop_reps=loop_reps)


def kernel(_want_trace: bool = False, **inputs) -> np.ndarray:
    from concourse.bass_utils import run_bass_kernel_spmd

    x = np.ascontiguousarray(np.asarray(inputs["x"], np.float32))
    assert x.shape == (B, T, H)

    if not _fast_path_ok(inputs):
        return _numpy_fallback(inputs)

    basis_m, basis_y, v0, scal = _host_pack(inputs)

    if "prog" not in _program_cache:
        _program_cache["prog"] = _timing_build()
    nc = _program_cache["prog"]

    core_ids = list(range(8))
    in_maps = [
        {
            "xb": _core_x_window(x, c),
            "basis_m": basis_m,
            "basis_y": basis_y,
            "v0": v0,
            "scal": scal,
        }
        for c in core_ids
    ]
    res = run_bass_kernel_spmd(nc, in_maps, core_ids, trace=_want_trace)
    out = np.empty((B, T, H), np.float32)
    for c in core_ids:
        ybc = res.results[c]["yb"].reshape(B, SEG, H)
        out[:, c * SEG : (c + 1) * SEG] = ybc
    if _want_trace:
        kernel._last_results = res
    return out
